# revision 4
# baseline (speedup 1.0000x reference)
"""Trainium2 Bass kernel for nn_Interaction (leaky-softmax dual attention).

Math (per graph): h_m = Gm@Wm, h_p = Gp@Wp, s = (Gm@Wm@bm) (+) (Gp@Wp@bp),
W = exp(leaky(s)), attn = W/rowsum, out_m = elu(attn@h_p), out_p = elu(attn^T@h_m).

Decomposition used on-device (exact):
  leaky(s) = 0.2*s + 0.8*relu(s);  W = du_n * dv_m * Wt,  Wt = exp(0.8*relu(s))
  W'B[m,n] = dv_m*Wt  (orientation [m_p, n_f]; ACT bias = 0.2*v per-partition)
  Zt[n] = sum_m W'B[m,n]  (ones-matmul)  == rowsum/du_n
  out_m^T[e,n] = sum_m h_p[m,e]*W'B[m,n];  out_m = elu(out_m^T * (1/Zt_n))
  W A[n,m] = Wt (orientation [n_p, m_f]);  h_mZ = h_m/Zt_n
  out_p^T[e,m] = sum_n h_mZ[n,e]*WA[n,m];  out_p = elu(out_p^T * dv_m)

Sharding: batch 16 graphs -> 8 cores x 2 graphs (pure data parallel).
"""
import numpy as np
from contextlib import ExitStack

B, N, M, D = 16, 1024, 2048, 128
NCORES = 8
GPC = B // NCORES  # graphs per core
NT, MT = N // 128, M // 128

_cache = {}


def _split_multiwaits(nc):
    """Old-walrus compat: an instruction may carry at most one sem wait."""
    import concourse.mybir as mybir
    n_fixed = 0
    for f in nc.m.functions:
        for blk in f.blocks:
            new_list = []
            for inst in blk.instructions:
                si = inst.sync_info
                waits = list(si.on_wait) if (si and si.on_wait) else []
                if len(waits) > 1:
                    keep = waits[-1]
                    for w in waits[:-1]:
                        nop = mybir.InstNoOp(
                            name=f"{inst.name}_wsplit{n_fixed}",
                            ins=[], outs=[], engine=inst.engine,
                            sync_info=mybir.SyncInfo(on_wait=[w], on_update=[]),
                        )
                        new_list.append(nop)
                        n_fixed += 1
                    si.on_wait = [keep]
                new_list.append(inst)
            blk.instructions = new_list
    return n_fixed


def _bcast_ap(dram_ap, parts):
    import concourse.bass as bass
    return bass.AP(tensor=dram_ap.tensor, offset=dram_ap.offset,
                   ap=[[0, parts]] + list(dram_ap.ap[1:]))


def _build():
    import concourse.bass as bass
    import concourse.tile as tile
    from concourse import mybir

    f32 = mybir.dt.float32
    f32r = mybir.dt.float32r
    Exp = mybir.ActivationFunctionType.Exp
    ADD, MAX, MIN, MULT = (mybir.AluOpType.add, mybir.AluOpType.max,
                           mybir.AluOpType.min, mybir.AluOpType.mult)

    nc = bass.Bass(trn_type="TRN2", target_bir_lowering=False, debug=False,
                   num_devices=NCORES)
    gmT = nc.dram_tensor("gmT", [GPC, 128, N], f32, kind="ExternalInput").ap()
    gpT = nc.dram_tensor("gpT", [GPC, 128, M], f32, kind="ExternalInput").ap()
    Wm = nc.dram_tensor("Wm", [128, 128], f32, kind="ExternalInput").ap()
    Wp = nc.dram_tensor("Wp", [128, 128], f32, kind="ExternalInput").ap()
    WmT = nc.dram_tensor("WmT", [128, 128], f32, kind="ExternalInput").ap()
    WpT = nc.dram_tensor("WpT", [128, 128], f32, kind="ExternalInput").ap()
    bm = nc.dram_tensor("bm", [128, 1], f32, kind="ExternalInput").ap()
    bp = nc.dram_tensor("bp", [128, 1], f32, kind="ExternalInput").ap()
    omT = nc.dram_tensor("omT", [GPC, 128, N], f32, kind="ExternalOutput").ap()
    opT = nc.dram_tensor("opT", [GPC, 128, M], f32, kind="ExternalOutput").ap()

    with tile.TileContext(nc) as tc, ExitStack() as ctx:
        P = ctx.enter_context  # shorthand
        singles = P(tc.tile_pool(name="singles", bufs=1))
        gio = P(tc.tile_pool(name="gio", bufs=1))
        hpool = P(tc.tile_pool(name="hpool", bufs=1))
        rp = P(tc.tile_pool(name="rp", bufs=2))
        wp = P(tc.tile_pool(name="wpool", bufs=3))
        fin = P(tc.tile_pool(name="fin", bufs=1))
        rows = P(tc.tile_pool(name="rows", bufs=1))
        dr = P(tc.tile_pool(name="dr", bufs=2, space="DRAM"))
        ps_bank = P(tc.tile_pool(name="ps_bank", bufs=2, space="PSUM"))
        ps_om = P(tc.tile_pool(name="ps_om", bufs=1, space="PSUM"))
        ps_op = P(tc.tile_pool(name="ps_op", bufs=1, space="PSUM"))

        # ---- per-core constants ----
        Wm_sb = singles.tile([128, 128], f32)
        Wp_sb = singles.tile([128, 128], f32)
        WmT_sb = singles.tile([128, 128], f32)
        WpT_sb = singles.tile([128, 128], f32)
        bm_sb = singles.tile([128, 1], f32)
        bp_sb = singles.tile([128, 1], f32)
        for dst, src in ((Wm_sb, Wm), (Wp_sb, Wp), (WmT_sb, WmT),
                         (WpT_sb, WpT), (bm_sb, bm), (bp_sb, bp)):
            nc.sync.dma_start(dst[:], src)
        ones_r = singles.tile([128, 1], f32r)
        onesf = singles.tile([128, 1], f32)
        nc.vector.memset(onesf[:], 1.0)
        nc.vector.tensor_copy(ones_r[:], onesf[:])
        # w~ = W @ b  (via lhsT = W^T)
        wps = ps_bank.tile([128, 512], f32, tag="bank")
        nc.tensor.matmul(wps[:, 0:1], WmT_sb[:], bm_sb[:], start=True, stop=True)
        nc.tensor.matmul(wps[:, 1:2], WpT_sb[:], bp_sb[:], start=True, stop=True)
        wcols = singles.tile([128, 2], f32)
        nc.vector.tensor_copy(wcols[:], wps[:, 0:2])

        for g in range(GPC):
            # ---- load G^T ----
            gmT_sb = gio.tile([128, N], f32, tag="gmT")
            gpT_sb = gio.tile([128, M], f32, tag="gpT")
            nc.sync.dma_start(gmT_sb[:], gmT[g])
            nc.sync.dma_start(gpT_sb[:], gpT[g])

            # ---- score vectors u, v (fp32 exact) ----
            uv_row = rows.tile([1, N + M], f32, tag="uvrow")
            for j in range(N // 512):
                c = ps_bank.tile([128, 512], f32, tag="bank")
                nc.tensor.matmul(c[0:1, :], wcols[:, 0:1],
                                 gmT_sb[:, j*512:(j+1)*512], start=True, stop=True)
                nc.vector.tensor_copy(uv_row[:, j*512:(j+1)*512], c[0:1, :])
            for j in range(M // 512):
                c = ps_bank.tile([128, 512], f32, tag="bank")
                nc.tensor.matmul(c[0:1, :], wcols[:, 1:2],
                                 gpT_sb[:, j*512:(j+1)*512], start=True, stop=True)
                nc.vector.tensor_copy(uv_row[:, N+j*512:N+(j+1)*512], c[0:1, :])
            uv_dr = dr.tile([1, N + M], f32, tag="uvdr")
            nc.sync.dma_start(uv_dr[:], uv_row[:])
            # broadcast tiles + column views (via DRAM APs)
            bcastB = gio.tile([128, N], f32, tag="bcastB")
            nc.sync.dma_start(bcastB[:], _bcast_ap(uv_dr[0:1, 0:N], 128))
            bcastA = gio.tile([128, M], f32, tag="bcastA")
            nc.sync.dma_start(bcastA[:], _bcast_ap(uv_dr[0:1, N:N+M], 128))
            u_cols = rows.tile([128, NT], f32, tag="ucols")
            nc.sync.dma_start(u_cols[:], uv_dr[0:1, 0:N].rearrange("a (c p) -> p (a c)", p=128))
            v_cols = rows.tile([128, MT], f32, tag="vcols")
            nc.sync.dma_start(v_cols[:], uv_dr[0:1, N:N+M].rearrange("a (c p) -> p (a c)", p=128))
            b02v = rows.tile([128, MT], f32, tag="b02v")
            nc.vector.tensor_scalar_mul(b02v[:], v_cols[:], 0.2)

            # ---- h_p (f32r), h_m (f32) ----
            hp_sb = hpool.tile([128, M], f32r, tag="hp")
            hm_sb = hpool.tile([128, N], f32, tag="hm")
            for g4 in range(MT // 4):
                hps = ps_bank.tile([128, 512], f32, tag="bank")
                for k in range(4):
                    mc = g4 * 4 + k
                    nc.tensor.matmul(hps[:, k*128:(k+1)*128],
                                     gpT_sb[:, mc*128:(mc+1)*128], Wp_sb[:],
                                     start=True, stop=True)
                nc.vector.tensor_copy(hp_sb[:, g4*512:(g4+1)*512], hps[:])
            for g4 in range(NT // 4):
                hms = ps_bank.tile([128, 512], f32, tag="bank")
                for k in range(4):
                    ct = g4 * 4 + k
                    nc.tensor.matmul(hms[:, k*128:(k+1)*128],
                                     gmT_sb[:, ct*128:(ct+1)*128], Wm_sb[:],
                                     start=True, stop=True)
                nc.vector.tensor_copy(hm_sb[:, g4*512:(g4+1)*512], hms[:])

            # ---- orientation B: W'B = exp(0.8 relu(s) + 0.2 v_m) ----
            om_ps = ps_om.tile([128, N], f32, tag="omps")
            zt0 = ps_bank.tile([128, 512], f32, tag="bank")
            zt1 = ps_bank.tile([128, 512], f32, tag="bank")
            zt_chunks = (zt0, zt1)
            for mc in range(MT):
                rB = rp.tile([128, N], f32, tag="rB")
                nc.vector.tensor_scalar(rB[:], bcastB[:], v_cols[:, mc:mc+1], 0.0,
                                        op0=ADD, op1=MAX)
                wB = wp.tile([128, N], f32r, tag="wB")
                nc.scalar.activation(wB[:], rB[:], Exp,
                                     bias=b02v[:, mc:mc+1], scale=0.8)
                st, sp = (mc == 0), (mc == MT - 1)
                for j in range(N // 512):
                    nc.tensor.matmul(om_ps[:, j*512:(j+1)*512],
                                     hp_sb[:, mc*128:(mc+1)*128],
                                     wB[:, j*512:(j+1)*512], start=st, stop=sp)
                    nc.tensor.matmul(zt_chunks[j][0:1, :], ones_r[:],
                                     wB[:, j*512:(j+1)*512], start=st, stop=sp)

            # ---- Z chain ----
            zt_row = rows.tile([1, N], f32, tag="ztrow")
            nc.vector.tensor_copy(zt_row[:, 0:512], zt0[0:1, :])
            nc.vector.tensor_copy(zt_row[:, 512:1024], zt1[0:1, :])
            zt_dr = dr.tile([1, N], f32, tag="ztdr")
            nc.sync.dma_start(zt_dr[:], zt_row[:])
            zt_cols = rows.tile([128, NT], f32, tag="ztcols")
            nc.sync.dma_start(zt_cols[:], zt_dr[0:1, :].rearrange("a (c p) -> p (a c)", p=128))
            rz_cols = rows.tile([128, NT], f32, tag="rzcols")
            nc.vector.reciprocal(rz_cols[:], zt_cols[:])
            rz_dr = dr.tile([1, N], f32, tag="rzdr")
            nc.sync.dma_start(rz_dr[0:1, :].rearrange("a (c p) -> p (a c)", p=128), rz_cols[:])
            scaleM = fin.tile([128, N], f32, tag="scaleM")
            nc.sync.dma_start(scaleM[:], _bcast_ap(rz_dr[0:1, :], 128))
            # h_mZ = h_m / Zt  (f32r)
            hmz_sb = hpool.tile([128, N], f32r, tag="hmz")
            for ct in range(NT):
                nc.vector.tensor_scalar_mul(hmz_sb[:, ct*128:(ct+1)*128],
                                            hm_sb[:, ct*128:(ct+1)*128],
                                            rz_cols[:, ct:ct+1])

            # ---- out_m finalize: elu(y)=relu(y)+min(e^y,1)-1 ----
            ym = fin.tile([128, N], f32, tag="ym")
            nc.vector.tensor_tensor(ym[:], om_ps[:], scaleM[:], op=MULT)
            em = fin.tile([128, N], f32, tag="em")
            nc.scalar.activation(em[:], ym[:], Exp)
            nc.vector.tensor_scalar(em[:], em[:], 1.0, -1.0, op0=MIN, op1=ADD)
            rm = fin.tile([128, N], f32, tag="rm")
            nc.vector.tensor_scalar_max(rm[:], ym[:], 0.0)
            om_sb = fin.tile([128, N], f32, tag="om")
            nc.vector.tensor_tensor(om_sb[:], em[:], rm[:], op=ADD)
            nc.sync.dma_start(omT[g], om_sb[:])

            # ---- orientation A: WA = exp(0.8 relu(s)) ----
            op_ps = ps_op.tile([128, M], f32, tag="opps")
            for ct in range(NT):
                rA = rp.tile([128, M], f32, tag="rA")
                nc.vector.tensor_scalar(rA[:], bcastA[:], u_cols[:, ct:ct+1], 0.0,
                                        op0=ADD, op1=MAX)
                wA = wp.tile([128, M], f32r, tag="wA")
                nc.scalar.activation(wA[:], rA[:], Exp, scale=0.8)
                st, sp = (ct == 0), (ct == NT - 1)
                for j in range(M // 512):
                    nc.tensor.matmul(op_ps[:, j*512:(j+1)*512],
                                     hmz_sb[:, ct*128:(ct+1)*128],
                                     wA[:, j*512:(j+1)*512], start=st, stop=sp)

            # ---- out_p finalize ----
            dv_row = rows.tile([1, M], f32, tag="dvrow")
            nc.scalar.activation(dv_row[:], uv_row[0:1, N:N+M], Exp, scale=0.2)
            dv_dr = dr.tile([1, M], f32, tag="dvdr")
            nc.sync.dma_start(dv_dr[:], dv_row[:])
            scaleP = fin.tile([128, M], f32, tag="scaleP")
            nc.sync.dma_start(scaleP[:], _bcast_ap(dv_dr[0:1, :], 128))
            yp = fin.tile([128, M], f32, tag="yp")
            nc.vector.tensor_tensor(yp[:], op_ps[:], scaleP[:], op=MULT)
            ep = fin.tile([128, M], f32, tag="ep")
            nc.scalar.activation(ep[:], yp[:], Exp)
            nc.vector.tensor_scalar(ep[:], ep[:], 1.0, -1.0, op0=MIN, op1=ADD)
            rp_t = fin.tile([128, M], f32, tag="rpt")
            nc.vector.tensor_scalar_max(rp_t[:], yp[:], 0.0)
            op_sb = fin.tile([128, M], f32, tag="op")
            nc.vector.tensor_tensor(op_sb[:], ep[:], rp_t[:], op=ADD)
            nc.sync.dma_start(opT[g], op_sb[:])

    _split_multiwaits(nc)
    return nc


def kernel(graphs_feature_m, graphs_feature_p, W_m, W_p, b):
    from concourse.bass_utils import run_bass_kernel_spmd

    gm = np.ascontiguousarray(np.asarray(graphs_feature_m, dtype=np.float32))
    gp = np.ascontiguousarray(np.asarray(graphs_feature_p, dtype=np.float32))
    W_m = np.ascontiguousarray(np.asarray(W_m, dtype=np.float32))
    W_p = np.ascontiguousarray(np.asarray(W_p, dtype=np.float32))
    b = np.ascontiguousarray(np.asarray(b, dtype=np.float32))

    if "nc" not in _cache:
        _cache["nc"] = _build()
    nc = _cache["nc"]

    gmT = np.ascontiguousarray(gm.transpose(0, 2, 1)).reshape(NCORES, GPC, 128, N)
    gpT = np.ascontiguousarray(gp.transpose(0, 2, 1)).reshape(NCORES, GPC, 128, M)
    WmT = np.ascontiguousarray(W_m.T)
    WpT = np.ascontiguousarray(W_p.T)
    bm = np.ascontiguousarray(b[:128, :])
    bp = np.ascontiguousarray(b[128:, :])
    in_maps = [{"gmT": gmT[c], "gpT": gpT[c], "Wm": W_m, "Wp": W_p,
                "WmT": WmT, "WpT": WpT, "bm": bm, "bp": bp}
               for c in range(NCORES)]
    res = run_bass_kernel_spmd(nc, in_maps, core_ids=list(range(NCORES)))
    om = np.stack([r["omT"] for r in res.results])  # [8, GPC, 128, N]
    op = np.stack([r["opT"] for r in res.results])
    out_m = om.reshape(B, 128, N).transpose(0, 2, 1)
    out_p = op.reshape(B, 128, M).transpose(0, 2, 1)
    return np.ascontiguousarray(out_m), np.ascontiguousarray(out_p)
